# revision 1
# baseline (speedup 1.0000x reference)
"""Trainium2 Bass kernel for nn_DenoiseNet (langevin point-cloud denoiser).

Strategy (8 NeuronCores, SPMD, zero inter-core communication):
  - Shard over B(2) x 4 contiguous N-chunks of 4096 points, each core padded
    with a 64-point halo on both sides (dependency cone grows 3 pts/step,
    4 steps -> 12 needed). Global-edge clipping handled exactly via per-core
    weight data (zeros on interior cores), so one program runs on all cores.
  - Feature-major fp16 layout [128 feat, (k, n) cols]. Sliding-window gather
    and scatter_add become free-dim shifted access patterns; the scatter
    k-sum and the delta update ride matmul PSUM accumulation.
  - First score-net layer split: h0 = relu(W0g.T delta[n+off_k] + Gk[n]),
    with Gk = feat@W0[3:] + b0 + W0g.T(pcl_noisy[n+off_k] - pcl_noisy[n])
    precomputed once on device. Tracking delta (= pcl - pcl_noisy) keeps
    fp16 rounding off the large pcl values.
  - Each step runs as three software-pipelined passes (layer0 / block1 /
    block2+scatter) over k-PAIRED [128, 2, 512] tiles (3x2-bank PSUM pool):
    every relu/add covers two neighbor offsets in one instruction, halving
    elementwise instruction count. Elementwise ops are greedily
    load-balanced across ACT/DVE/GPSIMD.
  - Same-weight matmuls are emitted in runs of 4 (W0g x4, I128 x4, Wb1 x4,
    Wb2 x4) so the PE skips redundant weight reloads — measured ~92ns per
    avoided Ldweights on real silicon (~56us/rep; the cost model does not
    model weight loads at all). Runs of 8 via 2-block iterations measured
    SLOWER (248.7us/rep vs 218.6) — the coarser mm/ew phase barriers lose
    more overlap than longer runs save. Measured loop-slope 218553 ns/rep.
"""

import sys
import numpy as np

for _p in ("/opt/trn_rl_repo",):
    if _p not in sys.path:
        sys.path.insert(0, _p)

import concourse.bass as bass
import concourse.bacc as bacc
import concourse.tile as tile
from concourse import mybir
from concourse.bass_utils import run_bass_kernel_spmd

# ---- problem constants (hardcoded per harness contract) ----
B, N, D = 2, 16384, 3
F = 128
K = 4
OFF = [-2, -1, 0, 1]
STEPS, S0, DECAY = 4, 0.2, 0.95
CHUNK, HALO, GW = 4096, 64, 2
NP = CHUNK + 2 * HALO          # 4224 local points
NB = NP + 2 * GW               # 4228 buffer cols (with guards)
R4 = K * NP                    # 16896 (k,n) columns
N_CORES = 8

f32 = mybir.dt.float32
f16 = mybir.dt.float16
AF = mybir.ActivationFunctionType
ALU = mybir.AluOpType

_CH512 = [(c * 512, min(512, NP - c * 512)) for c in range((NP + 511) // 512)]
_CHNB = [(c * 512, min(512, NB - c * 512)) for c in range((NB + 511) // 512)]


def build_program(reps=1, loop_n=0):
    """Build the SPMD Bass/Tile program. Returns compiled Bacc module."""
    nc = bacc.Bacc("TRN2", target_bir_lowering=False, debug=False)

    def inp(name, shape, dt):
        return nc.dram_tensor(name, list(shape), dt, kind="ExternalInput").ap()

    d_pclT = inp("pclT", (4, NB), f16)
    d_delta0 = inp("delta0", (4, NB), f16)
    d_Wf1 = inp("Wf1", (3, F), f16)
    d_bf1 = inp("bf1", (F, 1), f32)
    d_WfW = inp("WfW", (F, F), f16)
    d_bg = inp("bg", (F, 1), f32)
    d_W0g = inp("W0g", (3, F), f16)
    d_W0gn = inp("W0gn", (3, F), f16)
    d_I128 = inp("I128", (F, F), f16)
    d_Wb1 = inp("Wb1", (F, F), f16)
    d_Wb2 = inp("Wb2", (F, F), f16)
    d_bb1 = inp("bb1", (F, 1), f32)
    d_bb2 = inp("bb2", (F, 1), f32)
    d_WoS = inp("WoS", (F, 3 * STEPS), f16)
    d_I4 = inp("I4aug", (4, 3 * STEPS), f16)
    d_eL = inp("eL", (F, 3 * STEPS), f16)
    d_eLn = inp("eLn", (F, 3 * STEPS), f16)
    d_eR = inp("eR", (F, 3 * STEPS), f16)
    d_eRn = inp("eRn", (F, 3 * STEPS), f16)
    d_flagL = inp("flagL", (4, 1), f32)
    d_flagR = inp("flagR", (4, 1), f32)
    d_out = nc.dram_tensor("outT", [4, CHUNK], f16, kind="ExternalOutput").ap()

    from contextlib import ExitStack
    with tile.TileContext(nc) as tc, ExitStack() as ctx:
        cpool = ctx.enter_context(tc.tile_pool(name="const", bufs=1))
        hpool = ctx.enter_context(tc.tile_pool(name="h", bufs=4))
        tpool = ctx.enter_context(tc.tile_pool(name="tiny", bufs=2))
        psp = ctx.enter_context(tc.tile_pool(name="ps", bufs=3, space="PSUM"))
        pspp = ctx.enter_context(tc.tile_pool(name="psP", bufs=2, space="PSUM"))
        h0pool = ctx.enter_context(tc.tile_pool(name="h0p", bufs=18))

        def load(dram, shape, dt, tag):
            t = cpool.tile(list(shape), dt, tag=tag)
            nc.sync.dma_start(t[:], dram[:])
            return t

        pclT = load(d_pclT, (4, NB), f16, "pclT")
        delta_a = load(d_delta0, (4, NB), f16, "delta_a")
        delta_b = load(d_delta0, (4, NB), f16, "delta_b")
        Wf1 = load(d_Wf1, (3, F), f16, "Wf1")
        bf1 = load(d_bf1, (F, 1), f32, "bf1")
        WfW = load(d_WfW, (F, F), f16, "WfW")
        bg = load(d_bg, (F, 1), f32, "bg")
        W0g = load(d_W0g, (3, F), f16, "W0g")
        W0gn = load(d_W0gn, (3, F), f16, "W0gn")
        I128 = load(d_I128, (F, F), f16, "I128")
        Wb1 = load(d_Wb1, (F, F), f16, "Wb1")
        Wb2 = load(d_Wb2, (F, F), f16, "Wb2")
        bb1 = load(d_bb1, (F, 1), f32, "bb1")
        bb2 = load(d_bb2, (F, 1), f32, "bb2")
        WoS = load(d_WoS, (F, 3 * STEPS), f16, "WoS")
        I4 = load(d_I4, (4, 3 * STEPS), f16, "I4")
        eL = load(d_eL, (F, 3 * STEPS), f16, "eL")
        eLn = load(d_eLn, (F, 3 * STEPS), f16, "eLn")
        eR = load(d_eR, (F, 3 * STEPS), f16, "eR")
        eRn = load(d_eRn, (F, 3 * STEPS), f16, "eRn")
        flagL = load(d_flagL, (4, 1), f32, "flagL")
        flagR = load(d_flagR, (4, 1), f32, "flagR")

        Gk = cpool.tile([F, R4], f16, tag="Gk")
        h2_a = cpool.tile([F, R4], f16, tag="h2_a")
        h2_b = cpool.tile([F, R4], f16, tag="h2_b")
        A0e = cpool.tile([F, NB], f16, tag="A0e")
        G0 = cpool.tile([F, NP], f16, tag="G0")
        # 3-dim [F, k, n] views of the flat k-major tensors, for k-paired ops
        h2v_a = h2_a[:, 0:R4].rearrange("p (k m) -> p k m", k=K)
        h2v_b = h2_b[:, 0:R4].rearrange("p (k m) -> p k m", k=K)

        # greedy engine balancer for elementwise work
        load_ns = {"ACT": 0.0, "DVE": 0.0, "GP": 0.0}

        def pick(cands):
            eng, cost, fn = min(cands, key=lambda c: load_ns[c[0]] + c[1])
            load_ns[eng] += cost
            fn()

        def relu_op(dst, src, fd, bias=None):
            # psum -> sbuf relu, optional per-partition bias
            def on_act():
                nc.scalar.activation(dst, src, AF.Relu,
                                     bias=(bias[:, :] if bias is not None else 0.0))
            def on_dve():
                if bias is not None:
                    nc.vector.tensor_scalar(dst, src, bias[:, :], 0.0, ALU.add, ALU.max)
                else:
                    nc.vector.tensor_scalar_max(dst, src, 0.0)
            pick([("ACT", (fd + 212) * 0.833 + 16, on_act),
                  ("DVE", (fd + 60) * 1.042 + 15, on_dve)])

        def copy_op(dst, src, fd):
            def on_act():
                nc.scalar.activation(dst, src, AF.Copy)
            def on_dve():
                nc.vector.tensor_copy(dst, src)
            pick([("ACT", (fd + 212) * 0.833 + 16, on_act),
                  ("DVE", (fd + 60) * 1.042 + 15, on_dve)])

        def add_op(dst, a, b, fd):
            def on_dve():
                nc.vector.tensor_add(dst, a, b)
            def on_gp():
                nc.gpsimd.tensor_add(dst, a, b)
            pick([("DVE", (fd / 2 + 52) * 1.042 + 15, on_dve),
                  ("GP", fd * 2.3, on_gp)])

        # one column at the k=2/k=3 boundary is read (as cone garbage) by the
        # interleaved scatter before any tile writes it on step 0
        nc.vector.memset(h2_a[:, 3 * NP - 1:3 * NP], 0.0)
        nc.vector.memset(h2_b[:, 3 * NP - 1:3 * NP], 0.0)

        # ---------------- preamble: A0e, G0, Gk ----------------
        # chunk-paired [F,2,512] tiles: one crossing per 1024 cols
        def _pairs(total):
            out = []
            c = 0
            while c < total:
                e0 = min(512, total - c)
                e1 = min(512, total - c - e0)
                out.append((c, e0, e1))
                c += e0 + e1
            return out

        for c0, e0, e1 in _pairs(NB):
            ps = psp.tile([F, 2, 512], f32, tag="ps", name="ps")
            for h, (cc, ee) in enumerate(((c0, e0), (c0 + e0, e1))):
                if ee:
                    nc.tensor.matmul(ps[:, h, :ee], W0g[:, :],
                                     pclT[0:3, cc:cc + ee], start=True, stop=True)
            if e1 == 512:
                copy_op(A0e[:, c0:c0 + 1024].rearrange("p (b m) -> p b m", b=2),
                        ps[:, :, :], 1024)
            else:
                copy_op(A0e[:, c0:c0 + e0], ps[:, 0, :e0], e0)
                if e1:
                    copy_op(A0e[:, c0 + e0:c0 + e0 + e1], ps[:, 1, :e1], e1)
        for c0, e0, e1 in _pairs(NP):
            ps = psp.tile([F, 2, 512], f32, tag="ps", name="ps")
            for h, (cc, ee) in enumerate(((c0, e0), (c0 + e0, e1))):
                if ee:
                    nc.tensor.matmul(ps[:, h, :ee], Wf1[:, :],
                                     pclT[0:3, GW + cc:GW + cc + ee],
                                     start=True, stop=True)
            hf = hpool.tile([F, 2, 512], f16, tag="h0")
            if e1 == 512:
                nc.scalar.activation(hf[:, :, :], ps[:, :, :], AF.Relu, bias=bf1[:, :])
            else:
                nc.scalar.activation(hf[:, 0, :e0], ps[:, 0, :e0], AF.Relu,
                                     bias=bf1[:, :])
            ps2 = psp.tile([F, 2, 512], f32, tag="ps", name="ps")
            for h, (cc, ee) in enumerate(((c0, e0), (c0 + e0, e1))):
                if ee:
                    nc.tensor.matmul(ps2[:, h, :ee], WfW[:, :], hf[:, h, :ee],
                                     start=True, stop=False)
            for h, (cc, ee) in enumerate(((c0, e0), (c0 + e0, e1))):
                if ee:
                    nc.tensor.matmul(ps2[:, h, :ee], W0gn[:, :],
                                     pclT[0:3, GW + cc:GW + cc + ee],
                                     start=False, stop=True)
            if e1 == 512:
                nc.scalar.activation(G0[:, c0:c0 + 1024]
                                     .rearrange("p (b m) -> p b m", b=2),
                                     ps2[:, :, :], AF.Identity, bias=bg[:, :])
            else:
                nc.scalar.activation(G0[:, c0:c0 + e0], ps2[:, 0, :e0],
                                     AF.Identity, bias=bg[:, :])
        for k in range(K):
            for c0, e0, e1 in _pairs(NP):
                ext = e0 + e1
                add_op(Gk[:, k * NP + c0:k * NP + c0 + ext], G0[:, c0:c0 + ext],
                       A0e[:, GW + OFF[k] + c0:GW + OFF[k] + c0 + ext], ext)

        # ---------------- langevin steps ----------------
        from functools import partial

        def emit_rep(final_rep):
            sched = []
            for step in range(STEPS):
                d_in = delta_a if step % 2 == 0 else delta_b
                d_out_t = delta_b if step % 2 == 0 else delta_a
                h2 = h2_a if step % 2 == 0 else h2_b
                h2v = h2v_a if step % 2 == 0 else h2v_b
                final = (step == STEPS - 1) and final_rep
                s3 = slice(3 * step, 3 * step + 3)
                h0s = {}

                def emit_passA(cb, d_in=d_in, h0s=h0s):
                    c0, fd = _CH512[cb]
                    pss = [psp.tile([F, 2, 512], f32, tag="ps", name="ps")
                           for _ in range(2)]
                    for kh in range(2):
                        for j in range(2):
                            k = 2 * kh + j
                            nc.tensor.matmul(
                                pss[kh][:, j, :fd], W0g[:, :],
                                d_in[0:3, GW + OFF[k] + c0:GW + OFF[k] + c0 + fd],
                                start=True, stop=False)
                    for kh in range(2):
                        for j in range(2):
                            k = 2 * kh + j
                            nc.tensor.matmul(pss[kh][:, j, :fd], I128[:, :],
                                             Gk[:, k * NP + c0:k * NP + c0 + fd],
                                             start=False, stop=True)
                    for kh in range(2):
                        h0 = h0pool.tile([F, 2, 512], f16, tag="h0")
                        relu_op(h0[:, :, :fd], pss[kh][:, :, :fd], 2 * fd)
                        h0s[(kh, cb)] = h0

                def emit_passB(cb, h0s=h0s, h2v=h2v):
                    c0, fd = _CH512[cb]
                    pss = [psp.tile([F, 2, 512], f32, tag="ps", name="ps")
                           for _ in range(2)]
                    for kh in range(2):
                        h0 = h0s[(kh, cb)]
                        for j in range(2):
                            nc.tensor.matmul(pss[kh][:, j, :fd], Wb1[:, :],
                                             h0[:, j, :fd], start=True, stop=True)
                    for kh in range(2):
                        h0 = h0s[(kh, cb)]
                        r1 = hpool.tile([F, 2, 512], f16, tag="r1")
                        relu_op(r1[:, :, :fd], pss[kh][:, :, :fd], 2 * fd, bias=bb1)
                        add_op(h2v[:, 2 * kh:2 * kh + 2, c0:c0 + fd],
                               h0[:, :, :fd], r1[:, :, :fd], 2 * fd)

                def emit_passC(cb, h2=h2, h2v=h2v):
                    c0, fd = _CH512[cb]
                    pss = [psp.tile([F, 2, 512], f32, tag="ps", name="ps")
                           for _ in range(2)]
                    for kh in range(2):
                        for j in range(2):
                            k = 2 * kh + j
                            nc.tensor.matmul(pss[kh][:, j, :fd], Wb2[:, :],
                                             h2[:, k * NP + c0:k * NP + c0 + fd],
                                             start=True, stop=True)
                    for kh in range(2):
                        r2 = hpool.tile([F, 2, 512], f16, tag="r2")
                        relu_op(r2[:, :, :fd], pss[kh][:, :, :fd], 2 * fd, bias=bb2)
                        add_op(h2v[:, 2 * kh:2 * kh + 2, c0:c0 + fd],
                               h2v[:, 2 * kh:2 * kh + 2, c0:c0 + fd],
                               r2[:, :, :fd], 2 * fd)

                def mirror_fix(flag, src_l, dst0, ndst, d_out_t=d_out_t):
                    # mirror guards at global edges (flag=0 -> no-op on interior)
                    t = tpool.tile([4, 2], f16, tag="mir")
                    srcb = d_out_t[0:3, GW + src_l:GW + src_l + 1]\
                        .broadcast_to([3, ndst])
                    nc.vector.tensor_sub(t[0:3, 0:ndst], srcb,
                                         d_out_t[0:3, GW + dst0:GW + dst0 + ndst])
                    nc.vector.tensor_scalar_mul(t[0:3, 0:ndst], t[0:3, 0:ndst],
                                                flag[0:3, :])
                    nc.vector.tensor_add(d_out_t[0:3, GW + dst0:GW + dst0 + ndst],
                                         d_out_t[0:3, GW + dst0:GW + dst0 + ndst],
                                         t[0:3, 0:ndst])

                def emit_scatter(cb, d_in=d_in, d_out_t=d_out_t, h2=h2, s3=s3):
                    c0, fd = _CH512[cb]
                    ps = pspp.tile([4, 512], f32, tag="psP")
                    for k in range(K):
                        st = k * NP + c0 - OFF[k]
                        nc.tensor.matmul(ps[0:3, :fd], WoS[:, s3],
                                         h2[:, st:st + fd],
                                         start=(k == 0), stop=False)
                    if cb == 0:
                        pcol = ps[0:3, HALO:HALO + 1]
                        for col in (HALO, HALO + 1, NP + HALO):
                            nc.tensor.matmul(pcol, eL[:, s3], h2[:, col:col + 1],
                                             start=False, stop=False)
                        nc.tensor.matmul(pcol, eLn[:, s3],
                                         h2[:, 3 * NP + HALO - 1:3 * NP + HALO],
                                         start=False, stop=False)
                    if cb == len(_CH512) - 1:
                        lN = HALO + CHUNK - 1
                        pN = ps[0:3, lN - c0:lN - c0 + 1]
                        nc.tensor.matmul(pN, eR[:, s3], h2[:, 3 * NP + lN:3 * NP + lN + 1],
                                         start=False, stop=False)
                        for col in (lN + 2, NP + lN + 1):
                            nc.tensor.matmul(pN, eRn[:, s3], h2[:, col:col + 1],
                                             start=False, stop=False)
                        nc.tensor.matmul(ps[0:3, lN - 1 - c0:lN - c0], eRn[:, s3],
                                         h2[:, lN + 1:lN + 2], start=False, stop=False)
                    nc.tensor.matmul(ps[0:3, :fd], I4[:, s3],
                                     d_in[0:4, GW + c0:GW + c0 + fd],
                                     start=False, stop=True)
                    nc.vector.tensor_copy(d_out_t[0:3, GW + c0:GW + c0 + fd], ps[0:3, :fd])
                    load_ns["DVE"] += (fd + 60) * 1.042 + 15

                nblk = len(_CH512)
                base = 9 * step
                for cb in range(nblk + 5):
                    g = base + cb
                    if cb < nblk:
                        sched.append((g, step, 0, partial(emit_passA, cb)))
                    if 0 <= cb - 2 < nblk:
                        sched.append((g, step, 1, partial(emit_passB, cb - 2)))
                    if 0 <= cb - 3 < nblk:
                        sched.append((g, step, 2, partial(emit_passC, cb - 3)))
                    if 0 <= cb - 5 < nblk:
                        sched.append((g, step, 3, partial(emit_scatter, cb - 5)))
                        if not final and cb - 5 == 0:
                            sched.append((g, step, 4, partial(
                                mirror_fix, flagL, HALO, HALO - 2, 2)))
                        if not final and cb - 5 == nblk - 1:
                            sched.append((g, step, 4, partial(
                                mirror_fix, flagR, HALO + CHUNK - 1,
                                HALO + CHUNK, 1)))
                if final:
                    def final_dma(d=d_out_t):
                        nc.sync.dma_start(
                            d_out[:, :], d[0:4, GW + HALO:GW + HALO + CHUNK])
                    sched.append((base + nblk + 5, step, 9, final_dma))

            for _, _, _, fn in sorted(sched, key=lambda t: t[:3]):
                fn()

        if loop_n:
            with tc.For_i(0, loop_n, 1):
                emit_rep(False)
            emit_rep(True)
        else:
            for rep in range(reps):
                emit_rep(rep == reps - 1)

    nc.compile()
    return nc


def host_prep(inputs):
    """Slice/transpose/pad inputs per core; build weight-variant constants."""
    pcl = np.asarray(inputs["pcl_noisy"], np.float32)
    Wf1 = np.asarray(inputs["Wf1"], np.float32)
    bf1 = np.asarray(inputs["bf1"], np.float32)
    Wf2 = np.asarray(inputs["Wf2"], np.float32)
    bf2 = np.asarray(inputs["bf2"], np.float32)
    W0 = np.asarray(inputs["W0"], np.float32)
    b0 = np.asarray(inputs["b0"], np.float32)
    Wb = np.asarray(inputs["Wb"], np.float32)
    bb = np.asarray(inputs["bb"], np.float32)
    Wo = np.asarray(inputs["Wo"], np.float32)
    bo = np.asarray(inputs["bo"], np.float32)

    W0g = W0[:3]
    WfW = Wf2 @ W0[3:]
    bg = bf2 @ W0[3:] + b0
    offs = np.arange(-(K - 1) // 2, (K - 1) // 2 + 1)
    nbr = np.clip(np.arange(N)[:, None] + offs, 0, N - 1).reshape(-1)
    c_global = np.bincount(nbr, minlength=N).astype(np.float32)

    svals = [S0 * DECAY ** i for i in range(STEPS)]
    WoS = np.concatenate([s * Wo for s in svals], axis=1)          # [128, 12]
    I4 = np.zeros((4, 3 * STEPS), np.float32)
    for i, s in enumerate(svals):
        blk = np.eye(4, 3, dtype=np.float32)
        blk[3, 0:3] = s * bo
        I4[:, 3 * i:3 * i + 3] = blk

    hf = np.float16
    shared = {
        "Wf1": Wf1.astype(hf), "bf1": bf1.reshape(F, 1),
        "WfW": WfW.astype(hf), "bg": bg.reshape(F, 1),
        "W0g": W0g.astype(hf), "W0gn": (-W0g).astype(hf),
        "I128": np.eye(F, dtype=np.float32).astype(hf),
        "Wb1": Wb[0].astype(hf), "Wb2": Wb[1].astype(hf),
        "bb1": bb[0].reshape(F, 1), "bb2": bb[1].reshape(F, 1),
        "WoS": WoS.astype(hf),
        "I4aug": I4.astype(hf),
    }
    zeros_e = np.zeros((F, 3 * STEPS), np.float16)
    in_maps = []
    for core in range(N_CORES):
        b, ch = core // 4, core % 4
        g0 = ch * CHUNK - HALO
        idx = np.clip(np.arange(g0 - GW, g0 + NP + GW), 0, N - 1)
        pclT = np.empty((4, NB), np.float16)
        pclT[0:3] = pcl[b, idx].T.astype(np.float16)
        pclT[3] = 0.0
        delta0 = np.zeros((4, NB), np.float16)
        delta0[3, GW:GW + NP] = c_global[np.clip(np.arange(g0, g0 + NP), 0, N - 1)]
        isL, isR = ch == 0, ch == 3
        m = dict(shared)
        m["pclT"] = pclT
        m["delta0"] = delta0
        m["eL"] = (WoS.astype(hf) if isL else zeros_e)
        m["eLn"] = ((-WoS).astype(hf) if isL else zeros_e)
        m["eR"] = (WoS.astype(hf) if isR else zeros_e)
        m["eRn"] = ((-WoS).astype(hf) if isR else zeros_e)
        m["flagL"] = np.full((4, 1), 1.0 if isL else 0.0, np.float32)
        m["flagR"] = np.full((4, 1), 1.0 if isR else 0.0, np.float32)
        in_maps.append(m)
    return in_maps


_CACHED = {}


def _get_program(reps=1):
    if reps not in _CACHED:
        _CACHED[reps] = build_program(reps)
    return _CACHED[reps]


def kernel(**inputs):
    nc = _get_program(1)
    in_maps = host_prep(inputs)
    res = run_bass_kernel_spmd(nc, in_maps, list(range(N_CORES)))
    pcl = np.asarray(inputs["pcl_noisy"], np.float32)
    out = np.empty((B, N, D), np.float32)
    for core in range(N_CORES):
        b, ch = core // 4, core % 4
        sl = slice(ch * CHUNK, (ch + 1) * CHUNK)
        out[b, sl] = pcl[b, sl] + res.results[core]["outT"][0:3].T.astype(np.float32)
    return out



# revision 11
# speedup vs baseline: 1.1285x; 1.1285x over previous
"""Trainium2 Bass kernel for nn_DenoiseNet (langevin point-cloud denoiser).

V2: fp8 DoubleRow restructure of the previous fp16 kernel.

Strategy (8 NeuronCores, SPMD, zero inter-core communication):
  - Shard over B(2) x 4 contiguous N-chunks of 4096 points, 64-point halo
    (cone grows 3 pts/step, 4 steps -> 12 needed). Global-edge clipping via
    per-core weight data (zeros on interior cores).
  - All hot-loop matmul operands are fp8 e4m3; PE pair-matmuls run in
    DoubleRow perf mode (0.5 cycles/col, 2x) contracting 2x128:
      layer0:  psA = [W0gpad|I128] . [delta8 | Gk8]     (one DR matmul per k)
      passC:   psC = [Wb2|Wb2] . [h0 | r1]              (residual add rides PE)
      scatter: psS = sum_k WoS.(h0+r1+r2)(shifted) as 3 tensors x 2 k-pair
               DR matmuls (k-offsets are consecutive -> constant pair stride)
  - Both residual adds eliminated: h1/h2 never materialize. Scatter reads
    h0, r1, r2 separately; + delta update rides the same PSUM group (I4aug
    fp16 matmul). Elementwise per step is only the three PSUM relu drains
    (ACT/DVE, 1x regardless of dtype -> fp8 dst is free) + the d_out fp16
    copy + a Pool (GPSIMD) fp16->fp8 delta copy (Pool cannot touch PSUM).
  - fp8 state layout: one big M tile [128, delta_a|Gk|delta_b] so DoubleRow
    rhs pair APs (custom [part, 2(stride d), fd] access patterns) stay
    positive-stride; odd steps use a pair-reversed weight tile (W0gI8r).
    delta is carried fp16 (d_out) for the output/update path; fp8 copy is
    only a matmul operand (read noise, not accumulated state corruption).
  - Numerics checked against a faithful numpy emulator (ml_dtypes e4m3 ==
    TRN FP8_EXP4 bit-exact on ACT/DVE/Pool converts, verified on HW):
    rel err ~9.6e-3 vs 2e-2 budget. fp16 variant measured 2.5e-4.
"""

import sys
import numpy as np

for _p in ("/opt/trn_rl_repo",):
    if _p not in sys.path:
        sys.path.insert(0, _p)

import ml_dtypes
import concourse.bass as bass
import concourse.bacc as bacc
import concourse.tile as tile
from concourse import mybir
from concourse.bass_utils import run_bass_kernel_spmd

# ---- problem constants (hardcoded per harness contract) ----
B, N, D = 2, 16384, 3
F = 128
K = 4
OFF = [-2, -1, 0, 1]
STEPS, S0, DECAY = 4, 0.2, 0.95
CHUNK, HALO, GW = 4096, 64, 2
NP = CHUNK + 2 * HALO          # 4224 local points
NB = NP + 2 * GW               # 4228 buffer cols (with guards)
NPP = NP + 1                   # padded k-block stride: keeps every DoubleRow
                               # rhs pair stride 4-byte aligned (HW requires)
R4 = K * NPP                   # 16900 (k,n) columns incl 1 pad col per block
N_CORES = 8
# fp8 pair-buffer M regions: [delta_a | Gk | delta_b]
DA, GKo, DB = 0, NB, NB + R4
MB = 2 * NB + R4

f32 = mybir.dt.float32
f16 = mybir.dt.float16
f8 = mybir.dt.float8e4
AF = mybir.ActivationFunctionType
ALU = mybir.AluOpType
DR = mybir.MatmulPerfMode.DoubleRow

_CH512 = [(c * 512, min(512, NP - c * 512)) for c in range((NP + 511) // 512)]


def pap(base, stride, count):
    """Insert a [stride, count] dim after the partition dim of a 2-dim AP."""
    return bass.AP(tensor=base.tensor, offset=base.offset,
                   ap=[list(base.ap[0]), [int(stride), int(count)]]
                      + [list(d) for d in base.ap[1:]])


def build_program(reps=1, loop_n=0):
    """Build the SPMD Bass/Tile program. Returns compiled Bacc module."""
    nc = bacc.Bacc("TRN2", target_bir_lowering=False, debug=False)

    def inp(name, shape, dt):
        return nc.dram_tensor(name, list(shape), dt, kind="ExternalInput").ap()

    d_pclT = inp("pclT", (4, NB), f16)
    d_delta0 = inp("delta0", (4, NB), f16)
    d_Wf1 = inp("Wf1", (3, F), f16)
    d_bf1 = inp("bf1", (F, 1), f32)
    d_WfW = inp("WfW", (F, F), f16)
    d_bg = inp("bg", (F, 1), f32)
    d_W0g = inp("W0g", (3, F), f16)
    d_W0gn = inp("W0gn", (3, F), f16)
    d_W0gI8 = inp("W0gI8", (F, 2 * F), f8)
    d_W0gI8r = inp("W0gI8r", (F, 2 * F), f8)
    d_Wb1_8 = inp("Wb1_8", (F, F), f8)
    d_Wb2p8 = inp("Wb2p8", (F, 2 * F), f8)
    d_WoSp8 = inp("WoSp8", (F, 32 * STEPS), f8)
    d_I4 = inp("I4aug", (4, 3 * STEPS), f16)
    d_bb1 = inp("bb1", (F, 1), f32)
    d_bb2 = inp("bb2", (F, 1), f32)
    d_eL = inp("eL8", (F, 4 * STEPS), f8)
    d_eLn = inp("eLn8", (F, 4 * STEPS), f8)
    d_eR = inp("eR8", (F, 4 * STEPS), f8)
    d_eRn = inp("eRn8", (F, 4 * STEPS), f8)
    d_flagL = inp("flagL", (4, 1), f32)
    d_flagR = inp("flagR", (4, 1), f32)
    d_out = nc.dram_tensor("outT", [4, CHUNK], f16, kind="ExternalOutput").ap()

    from contextlib import ExitStack
    with tile.TileContext(nc) as tc, ExitStack() as ctx:
        cpool = ctx.enter_context(tc.tile_pool(name="const", bufs=1))
        hpool = ctx.enter_context(tc.tile_pool(name="h", bufs=4))
        tpool = ctx.enter_context(tc.tile_pool(name="tiny", bufs=2))
        psp = ctx.enter_context(tc.tile_pool(name="ps", bufs=3, space="PSUM"))
        pspp = ctx.enter_context(tc.tile_pool(name="psP", bufs=2, space="PSUM"))

        def load(dram, shape, dt, tag):
            t = cpool.tile(list(shape), dt, tag=tag)
            nc.sync.dma_start(t[:], dram[:])
            return t

        pclT = load(d_pclT, (4, NB), f16, "pclT")
        delta_a = load(d_delta0, (4, NB), f16, "delta_a")
        delta_b = load(d_delta0, (4, NB), f16, "delta_b")
        Wf1 = load(d_Wf1, (3, F), f16, "Wf1")
        bf1 = load(d_bf1, (F, 1), f32, "bf1")
        WfW = load(d_WfW, (F, F), f16, "WfW")
        bg = load(d_bg, (F, 1), f32, "bg")
        W0g = load(d_W0g, (3, F), f16, "W0g")
        W0gn = load(d_W0gn, (3, F), f16, "W0gn")
        W0gI8 = load(d_W0gI8, (F, 2 * F), f8, "W0gI8")
        W0gI8r = load(d_W0gI8r, (F, 2 * F), f8, "W0gI8r")
        Wb1_8 = load(d_Wb1_8, (F, F), f8, "Wb1_8")
        Wb2p8 = load(d_Wb2p8, (F, 2 * F), f8, "Wb2p8")
        WoSp8 = load(d_WoSp8, (F, 32 * STEPS), f8, "WoSp8")
        I4 = load(d_I4, (4, 3 * STEPS), f16, "I4")
        bb1 = load(d_bb1, (F, 1), f32, "bb1")
        bb2 = load(d_bb2, (F, 1), f32, "bb2")
        eL = load(d_eL, (F, 4 * STEPS), f8, "eL8")
        eLn = load(d_eLn, (F, 4 * STEPS), f8, "eLn8")
        eR = load(d_eR, (F, 4 * STEPS), f8, "eR8")
        eRn = load(d_eRn, (F, 4 * STEPS), f8, "eRn8")
        flagL = load(d_flagL, (4, 1), f32, "flagL")
        flagR = load(d_flagR, (4, 1), f32, "flagR")

        M = cpool.tile([F, MB], f8, tag="M")
        H_a = cpool.tile([F, 2, R4], f8, tag="H_a")
        H_b = cpool.tile([F, 2, R4], f8, tag="H_b")
        r2_a = cpool.tile([F, R4], f8, tag="r2_a")
        r2_b = cpool.tile([F, R4], f8, tag="r2_b")
        A0e = cpool.tile([F, NB], f16, tag="A0e")
        G0 = cpool.tile([F, NP], f16, tag="G0")

        # pair-view weights (dim1 = the DoubleRow k-tile pair)
        W0gI8v = pap(W0gI8[:, 0:F], F, 2)
        W0gI8rv = pap(W0gI8r[:, 0:F], F, 2)
        Wb2p8v = pap(Wb2p8[:, 0:F], F, 2)

        # greedy engine balancer for elementwise work
        load_ns = {"ACT": 0.0, "DVE": 0.0, "GP": 0.0}

        def pick(cands):
            eng, cost, fn = min(cands, key=lambda c: load_ns[c[0]] + c[1])
            load_ns[eng] += cost
            fn()

        def relu_op(dst, src, fd, bias=None):
            # psum -> sbuf relu, optional per-partition bias (dtype-agnostic)
            def on_act():
                nc.scalar.activation(dst, src, AF.Relu,
                                     bias=(bias[:, :] if bias is not None else 0.0))
            def on_dve():
                if bias is not None:
                    nc.vector.tensor_scalar(dst, src, bias[:, :], 0.0, ALU.add, ALU.max)
                else:
                    nc.vector.tensor_scalar_max(dst, src, 0.0)
            pick([("ACT", (fd + 212) * 0.833 + 16, on_act),
                  ("DVE", (fd + 60) * 1.042 + 15, on_dve)])

        def copy_op(dst, src, fd):
            def on_act():
                nc.scalar.activation(dst, src, AF.Copy)
            def on_dve():
                nc.vector.tensor_copy(dst, src)
            pick([("ACT", (fd + 212) * 0.833 + 16, on_act),
                  ("DVE", (fd + 60) * 1.042 + 15, on_dve)])

        def conv_op(dst, src, fd):
            # sbuf fp16 -> sbuf fp8 copy; Pool-eligible
            def on_act():
                nc.scalar.activation(dst, src, AF.Copy)
            def on_dve():
                nc.vector.tensor_copy(dst, src)
            def on_gp():
                nc.gpsimd.tensor_copy(dst, src)
            pick([("ACT", (fd + 212) * 0.833 + 16, on_act),
                  ("DVE", (fd + 60) * 1.042 + 15, on_dve),
                  ("GP", fd * 1.39 + 95, on_gp)])

        def add_op8(dst, a, b, fd):
            # fp16+fp16 -> fp8 dst (1x on DVE: 1-byte dst)
            def on_dve():
                nc.vector.tensor_add(dst, a, b)
            def on_gp():
                nc.gpsimd.tensor_add(dst, a, b)
            pick([("DVE", (fd + 60) * 1.042 + 15, on_dve),
                  ("GP", fd * 1.98 + 95, on_gp)])

        # zero fp8 delta regions (partitions 3..127 feed zero weight rows of
        # the layer0 pair matmul and must be finite; rows 0..2 start at
        # delta=0) and the one stale column each scatter reads before any
        # same-step drain writes it.
        nc.vector.memset(M[:, DA:DA + NB], 0.0)
        nc.gpsimd.memset(M[:, DB:DB + NB], 0.0)
        for t in (H_a, H_b):
            for k in range(K):
                nc.vector.memset(t[:, :, (k + 1) * NPP - 1:(k + 1) * NPP], 0.0)
        for t in (r2_a, r2_b):
            for k in range(K):
                nc.vector.memset(t[:, (k + 1) * NPP - 1:(k + 1) * NPP], 0.0)

        # ---------------- preamble: A0e, G0, Gk8 ----------------
        def _pairs(total):
            out = []
            c = 0
            while c < total:
                e0 = min(512, total - c)
                e1 = min(512, total - c - e0)
                out.append((c, e0, e1))
                c += e0 + e1
            return out

        for c0, e0, e1 in _pairs(NB):
            ps = psp.tile([F, 2, 512], f32, tag="ps", name="ps")
            for h, (cc, ee) in enumerate(((c0, e0), (c0 + e0, e1))):
                if ee:
                    nc.tensor.matmul(ps[:, h, :ee], W0g[:, :],
                                     pclT[0:3, cc:cc + ee], start=True, stop=True)
            if e1 == 512:
                copy_op(A0e[:, c0:c0 + 1024].rearrange("p (b m) -> p b m", b=2),
                        ps[:, :, :], 1024)
            else:
                copy_op(A0e[:, c0:c0 + e0], ps[:, 0, :e0], e0)
                if e1:
                    copy_op(A0e[:, c0 + e0:c0 + e0 + e1], ps[:, 1, :e1], e1)
        for c0, e0, e1 in _pairs(NP):
            ps = psp.tile([F, 2, 512], f32, tag="ps", name="ps")
            for h, (cc, ee) in enumerate(((c0, e0), (c0 + e0, e1))):
                if ee:
                    nc.tensor.matmul(ps[:, h, :ee], Wf1[:, :],
                                     pclT[0:3, GW + cc:GW + cc + ee],
                                     start=True, stop=True)
            hf = hpool.tile([F, 2, 512], f16, tag="h0")
            if e1 == 512:
                nc.scalar.activation(hf[:, :, :], ps[:, :, :], AF.Relu, bias=bf1[:, :])
            else:
                nc.scalar.activation(hf[:, 0, :e0], ps[:, 0, :e0], AF.Relu,
                                     bias=bf1[:, :])
            ps2 = psp.tile([F, 2, 512], f32, tag="ps", name="ps")
            for h, (cc, ee) in enumerate(((c0, e0), (c0 + e0, e1))):
                if ee:
                    nc.tensor.matmul(ps2[:, h, :ee], WfW[:, :], hf[:, h, :ee],
                                     start=True, stop=False)
            for h, (cc, ee) in enumerate(((c0, e0), (c0 + e0, e1))):
                if ee:
                    nc.tensor.matmul(ps2[:, h, :ee], W0gn[:, :],
                                     pclT[0:3, GW + cc:GW + cc + ee],
                                     start=False, stop=True)
            if e1 == 512:
                nc.scalar.activation(G0[:, c0:c0 + 1024]
                                     .rearrange("p (b m) -> p b m", b=2),
                                     ps2[:, :, :], AF.Identity, bias=bg[:, :])
            else:
                nc.scalar.activation(G0[:, c0:c0 + e0], ps2[:, 0, :e0],
                                     AF.Identity, bias=bg[:, :])
        for k in range(K):
            for c0, e0, e1 in _pairs(NP):
                ext = e0 + e1
                add_op8(M[:, GKo + k * NPP + c0:GKo + k * NPP + c0 + ext],
                        G0[:, c0:c0 + ext],
                        A0e[:, GW + OFF[k] + c0:GW + OFF[k] + c0 + ext], ext)

        # ---------------- langevin steps ----------------
        from functools import partial

        def emit_rep(final_rep):
            sched = []
            for step in range(STEPS):
                even = step % 2 == 0
                d_in = delta_a if even else delta_b
                d_out_t = delta_b if even else delta_a
                din8 = DA if even else DB
                dout8 = DB if even else DA
                W0v = W0gI8v if even else W0gI8rv
                H = H_a if even else H_b
                r2t = r2_a if even else r2_b
                final = (step == STEPS - 1) and final_rep
                s3 = slice(3 * step, 3 * step + 3)
                s4 = slice(4 * step, 4 * step + 3)
                sw = slice(32 * step, 32 * step + 3)

                def emit_passA(cb, din8=din8, W0v=W0v, H=H):
                    c0, fd = _CH512[cb]
                    pss = [psp.tile([F, 2, 512], f32, tag="ps", name="ps")
                           for _ in range(2)]
                    for kh in range(2):
                        for j in range(2):
                            k = 2 * kh + j
                            el0 = din8 + GW + OFF[k] + c0
                            dlt = (GKo + k * NPP + c0) - el0
                            if dlt < 0:   # odd steps: Gk first, delta second
                                el0 = GKo + k * NPP + c0
                                dlt = (din8 + GW + OFF[k] + c0) - el0
                            rhs = pap(M[:, el0:el0 + fd], dlt, 2)
                            nc.tensor.matmul(pss[kh][:, j, :fd], W0v, rhs,
                                             start=True, stop=True, perf_mode=DR)
                    for kh in range(2):
                        base = H[:, 0, 2 * kh * NPP + c0:2 * kh * NPP + c0 + fd]
                        relu_op(pap(base, NPP, 2), pss[kh][:, :, :fd], 2 * fd)

                def emit_passB(cb, H=H):
                    c0, fd = _CH512[cb]
                    pss = [psp.tile([F, 2, 512], f32, tag="ps", name="ps")
                           for _ in range(2)]
                    for kh in range(2):
                        for j in range(2):
                            k = 2 * kh + j
                            nc.tensor.matmul(
                                pss[kh][:, j, :fd], Wb1_8[:, :],
                                H[:, 0, k * NPP + c0:k * NPP + c0 + fd],
                                start=True, stop=True)
                    for kh in range(2):
                        base = H[:, 1, 2 * kh * NPP + c0:2 * kh * NPP + c0 + fd]
                        relu_op(pap(base, NPP, 2), pss[kh][:, :, :fd], 2 * fd,
                                bias=bb1)

                def emit_passC(cb, H=H, r2t=r2t):
                    c0, fd = _CH512[cb]
                    pss = [psp.tile([F, 2, 512], f32, tag="ps", name="ps")
                           for _ in range(2)]
                    for kh in range(2):
                        for j in range(2):
                            k = 2 * kh + j
                            nc.tensor.matmul(
                                pss[kh][:, j, :fd], Wb2p8v,
                                H[:, :, k * NPP + c0:k * NPP + c0 + fd],
                                start=True, stop=True, perf_mode=DR)
                    for kh in range(2):
                        base = r2t[:, 2 * kh * NPP + c0:2 * kh * NPP + c0 + fd]
                        relu_op(pap(base, NPP, 2), pss[kh][:, :, :fd], 2 * fd,
                                bias=bb2)

                def mirror_fix(flag, src_l, dst0, ndst, d_out_t=d_out_t):
                    # mirror guards at global edges (flag=0 -> no-op on interior)
                    t = tpool.tile([4, 2], f16, tag="mir")
                    srcb = d_out_t[0:3, GW + src_l:GW + src_l + 1]\
                        .broadcast_to([3, ndst])
                    nc.vector.tensor_sub(t[0:3, 0:ndst], srcb,
                                         d_out_t[0:3, GW + dst0:GW + dst0 + ndst])
                    nc.vector.tensor_scalar_mul(t[0:3, 0:ndst], t[0:3, 0:ndst],
                                                flag[0:3, :])
                    nc.vector.tensor_add(d_out_t[0:3, GW + dst0:GW + dst0 + ndst],
                                         d_out_t[0:3, GW + dst0:GW + dst0 + ndst],
                                         t[0:3, 0:ndst])

                def h3srcs(col, H=H, r2t=r2t):
                    return (H[:, 0, col:col + 1], H[:, 1, col:col + 1],
                            r2t[:, col:col + 1])

                def emit_scatter(cb, d_in=d_in, d_out_t=d_out_t, H=H, r2t=r2t,
                                 s3=s3, s4=s4, sw=sw, h3srcs=h3srcs):
                    c0, fd = _CH512[cb]
                    ps = pspp.tile([4, 512], f32, tag="psP")
                    WoV = pap(WoSp8[:, sw], 16, 2)
                    first = True
                    for kh in range(2):
                        st0 = 2 * kh * NPP + c0 - OFF[2 * kh]
                        for src in (H[:, 0, st0:st0 + fd],
                                    H[:, 1, st0:st0 + fd],
                                    r2t[:, st0:st0 + fd]):
                            nc.tensor.matmul(ps[0:3, :fd], WoV,
                                             pap(src, NPP - 1, 2),
                                             start=first, stop=False,
                                             perf_mode=DR)
                            first = False
                    if cb == 0:
                        pcol = ps[0:3, HALO:HALO + 1]
                        for col in (HALO, HALO + 1, NPP + HALO):
                            for src in h3srcs(col):
                                nc.tensor.matmul(pcol, eL[:, s4], src,
                                                 start=False, stop=False)
                        for src in h3srcs(3 * NPP + HALO - 1):
                            nc.tensor.matmul(pcol, eLn[:, s4], src,
                                             start=False, stop=False)
                    if cb == len(_CH512) - 1:
                        lN = HALO + CHUNK - 1
                        pN = ps[0:3, lN - c0:lN - c0 + 1]
                        for src in h3srcs(3 * NPP + lN):
                            nc.tensor.matmul(pN, eR[:, s4], src,
                                             start=False, stop=False)
                        for col in (lN + 2, NPP + lN + 1):
                            for src in h3srcs(col):
                                nc.tensor.matmul(pN, eRn[:, s4], src,
                                                 start=False, stop=False)
                        for src in h3srcs(lN + 1):
                            nc.tensor.matmul(ps[0:3, lN - 1 - c0:lN - c0],
                                             eRn[:, s4], src,
                                             start=False, stop=False)
                    nc.tensor.matmul(ps[0:3, :fd], I4[:, s3],
                                     d_in[0:4, GW + c0:GW + c0 + fd],
                                     start=False, stop=True)
                    copy_op(d_out_t[0:3, GW + c0:GW + c0 + fd], ps[0:3, :fd], fd)

                def emit_conv(cb, d_out_t=d_out_t, dout8=dout8):
                    c0, fd = _CH512[cb]
                    conv_op(M[0:3, dout8 + GW + c0:dout8 + GW + c0 + fd],
                            d_out_t[0:3, GW + c0:GW + c0 + fd], fd)

                nblk = len(_CH512)
                base = 9 * step
                for cb in range(nblk + 5):
                    g = base + cb
                    if cb < nblk:
                        sched.append((g, step, 0, partial(emit_passA, cb)))
                    if 0 <= cb - 2 < nblk:
                        sched.append((g, step, 1, partial(emit_passB, cb - 2)))
                    if 0 <= cb - 3 < nblk:
                        sched.append((g, step, 2, partial(emit_passC, cb - 3)))
                    if 0 <= cb - 5 < nblk:
                        sched.append((g, step, 3, partial(emit_scatter, cb - 5)))
                        if not final and cb - 5 == 0:
                            sched.append((g, step, 4, partial(
                                mirror_fix, flagL, HALO, HALO - 2, 2)))
                        if not final and cb - 5 == nblk - 1:
                            sched.append((g, step, 4, partial(
                                mirror_fix, flagR, HALO + CHUNK - 1,
                                HALO + CHUNK, 1)))
                        if not final:
                            sched.append((g, step, 5, partial(emit_conv, cb - 5)))
                if final:
                    def final_dma(d=d_out_t):
                        nc.sync.dma_start(
                            d_out[:, :], d[0:4, GW + HALO:GW + HALO + CHUNK])
                    sched.append((base + nblk + 5, step, 9, final_dma))

            for _, _, _, fn in sorted(sched, key=lambda t: t[:3]):
                fn()

        if loop_n:
            with tc.For_i(0, loop_n, 1):
                emit_rep(False)
            emit_rep(True)
        else:
            for rep in range(reps):
                emit_rep(rep == reps - 1)

    nc.compile()
    return nc


def host_prep(inputs):
    """Slice/transpose/pad inputs per core; build weight-variant constants."""
    pcl = np.asarray(inputs["pcl_noisy"], np.float32)
    Wf1 = np.asarray(inputs["Wf1"], np.float32)
    bf1 = np.asarray(inputs["bf1"], np.float32)
    Wf2 = np.asarray(inputs["Wf2"], np.float32)
    bf2 = np.asarray(inputs["bf2"], np.float32)
    W0 = np.asarray(inputs["W0"], np.float32)
    b0 = np.asarray(inputs["b0"], np.float32)
    Wb = np.asarray(inputs["Wb"], np.float32)
    bb = np.asarray(inputs["bb"], np.float32)
    Wo = np.asarray(inputs["Wo"], np.float32)
    bo = np.asarray(inputs["bo"], np.float32)

    hf = np.float16
    f8np = ml_dtypes.float8_e4m3

    W0g = W0[:3]
    WfW = Wf2 @ W0[3:]
    bg = bf2 @ W0[3:] + b0
    offs = np.arange(-(K - 1) // 2, (K - 1) // 2 + 1)
    nbr = np.clip(np.arange(N)[:, None] + offs, 0, N - 1).reshape(-1)
    c_global = np.bincount(nbr, minlength=N).astype(np.float32)

    svals = [S0 * DECAY ** i for i in range(STEPS)]
    WoS = np.concatenate([s * Wo for s in svals], axis=1)          # [128, 12]
    I4 = np.zeros((4, 3 * STEPS), np.float32)
    for i, s in enumerate(svals):
        blk = np.eye(4, 3, dtype=np.float32)
        blk[3, 0:3] = s * bo
        I4[:, 3 * i:3 * i + 3] = blk

    W0gpad = np.zeros((F, F), np.float32)
    W0gpad[:3] = W0g
    I128 = np.eye(F, dtype=np.float32)
    # WoS pair layout padded to 16-byte pair stride (DoubleRow Ldweights
    # requires 16B-aligned pair strides): per step s, cols 32s..32s+2 and
    # 32s+16..32s+18 both hold WoS[:, 3s:3s+3]
    wospad = np.zeros((F, 32 * STEPS), np.float32)
    for _s in range(STEPS):
        wospad[:, 32 * _s:32 * _s + 3] = WoS[:, 3 * _s:3 * _s + 3]
        wospad[:, 32 * _s + 16:32 * _s + 19] = WoS[:, 3 * _s:3 * _s + 3]
    wospad = wospad.astype(f8np)

    def epad(w):
        # edge weights padded to 4 cols/step (aligned fp8 offsets)
        e = np.zeros((F, 4 * STEPS), np.float32)
        for _s in range(STEPS):
            e[:, 4 * _s:4 * _s + 3] = w[:, 3 * _s:3 * _s + 3]
        return e.astype(f8np)

    def pair8(a, b):
        return np.stack([a, b], axis=1).reshape(a.shape[0], -1).astype(f8np)

    shared = {
        "Wf1": Wf1.astype(hf), "bf1": bf1.reshape(F, 1),
        "WfW": WfW.astype(hf), "bg": bg.reshape(F, 1),
        "W0g": W0g.astype(hf), "W0gn": (-W0g).astype(hf),
        "W0gI8": pair8(W0gpad, I128),
        "W0gI8r": pair8(I128, W0gpad),
        "Wb1_8": Wb[0].astype(f8np),
        "Wb2p8": pair8(Wb[1], Wb[1]),
        "WoSp8": wospad,
        "I4aug": I4.astype(hf),
        "bb1": bb[0].reshape(F, 1), "bb2": bb[1].reshape(F, 1),
    }
    zeros_e = np.zeros((F, 4 * STEPS), f8np)
    in_maps = []
    for core in range(N_CORES):
        b, ch = core // 4, core % 4
        g0 = ch * CHUNK - HALO
        idx = np.clip(np.arange(g0 - GW, g0 + NP + GW), 0, N - 1)
        pclT = np.empty((4, NB), np.float16)
        pclT[0:3] = pcl[b, idx].T.astype(np.float16)
        pclT[3] = 0.0
        delta0 = np.zeros((4, NB), np.float16)
        delta0[3, GW:GW + NP] = c_global[np.clip(np.arange(g0, g0 + NP), 0, N - 1)]
        isL, isR = ch == 0, ch == 3
        m = dict(shared)
        m["pclT"] = pclT
        m["delta0"] = delta0
        m["eL8"] = (epad(WoS) if isL else zeros_e)
        m["eLn8"] = (epad(-WoS) if isL else zeros_e)
        m["eR8"] = (epad(WoS) if isR else zeros_e)
        m["eRn8"] = (epad(-WoS) if isR else zeros_e)
        m["flagL"] = np.full((4, 1), 1.0 if isL else 0.0, np.float32)
        m["flagR"] = np.full((4, 1), 1.0 if isR else 0.0, np.float32)
        in_maps.append(m)
    return in_maps


_CACHED = {}


def _get_program(reps=1):
    if reps not in _CACHED:
        _CACHED[reps] = build_program(reps)
    return _CACHED[reps]


def kernel(**inputs):
    nc = _get_program(1)
    in_maps = host_prep(inputs)
    res = run_bass_kernel_spmd(nc, in_maps, list(range(N_CORES)))
    pcl = np.asarray(inputs["pcl_noisy"], np.float32)
    out = np.empty((B, N, D), np.float32)
    for core in range(N_CORES):
        b, ch = core // 4, core % 4
        sl = slice(ch * CHUNK, (ch + 1) * CHUNK)
        out[b, sl] = pcl[b, sl] + res.results[core]["outT"][0:3].T.astype(np.float32)
    return out
